# revision 11
# baseline (speedup 1.0000x reference)
"""GPT-2 forward on 8 TRN2 NeuronCores — feature-major context-parallel Bass/Tile kernel.

Sharding: 4 sequences x 2 cores each (strided interleave: core 2b+p owns tokens
of sequence b at global positions {2u+p}). Per layer each pair AllGathers its
LN1 output aT (bf16, feature-major, E*T = 768KB) and every core computes k/v
for the FULL sequence locally — cheaper than exchanging k+v and it removes the
small-descriptor re-interleave unpack entirely. Key/value blocks stay PAR-major
(block (p,c) = par p's local tokens [128c,128c+128)); causality is encoded in
two host-built diagonal masks.

Residual h is FEATURE-major f32 in SBUF for the whole kernel: no DMA-transposes.
LayerNorm stats are computed with PE ones-matmuls over bf16 copies of h (sum)
and h^2 (sumsq); rstd = exp(-0.5*ln(var+eps)) so the scalar engine stays in the
natural_log_exp table set shared with the attention exp (2 table switches per
layer, for gelu, same as the token-major version). Mean/rstd rows are broadcast
to [128,T] with rank-1 PE matmuls (ones / -1/E columns) and the normalize TTs
read them straight from PSUM.

Attention: scores for head pairs are packed into disjoint PE row groups
(K=64 at base partitions 0/64) writing one 2-bank PSUM tile, exp'd in a single
ACT op, masked in a single DVE op; av is causally trimmed; softmax denominators
come from a ones-column appended to v. Odd-head outputs are normalized at
partitions 0-63 and shifted to 64-127 with one small SBUF-SBUF DMA per pair.

LN gamma/beta and the 1/sqrt(D) scale are folded into weights on the host;
biases that are identically zero are skipped at build time.
"""
import sys, os
sys.path.insert(0, '/opt/trn_rl_repo')
import numpy as np
import ml_dtypes
import concourse.bass as bass
import concourse.mybir as mybir
from concourse import bacc
from concourse.bass_utils import run_bass_kernel_spmd
from concourse.tile import TileContext

F32 = mybir.dt.float32
BF16 = mybir.dt.bfloat16
AF = mybir.ActivationFunctionType
ALU = mybir.AluOpType
BF = ml_dtypes.bfloat16


def cfg_full():
    return dict(B=4, S=1024, L=12, H=12, D=64, F=3072, V=50257)


def cfg_mini():
    return dict(B=4, S=256, L=2, H=2, D=64, F=256, V=640)


def derived(c):
    d = dict(c)
    d['E'] = c['H'] * c['D']
    d['T'] = c['S'] // 2          # local tokens per core
    d['QCH'] = d['T'] // 128      # query/token chunks (T/128)
    d['KCH'] = c['S'] // 128      # global key chunks
    d['ECH'] = d['E'] // 128      # embed chunks
    d['FCH'] = c['F'] // 128      # mlp hidden chunks
    d['VNC'] = (c['V'] + 511) // 512  # lm-head n-chunks
    assert d['T'] % 128 == 0 and d['E'] % 128 == 0 and c['F'] % 128 == 0
    return d


def build(c, has_bias):
    """has_bias: dict of bools (qkv, v, proj, fc, fc2, lm) — ops skipped when zero."""
    d = derived(c)
    T, E, H, D, F, V, L = d['T'], d['E'], c['H'], c['D'], c['F'], c['V'], c['L']
    QCH, KCH, ECH, FCH, VNC = d['QCH'], d['KCH'], d['ECH'], d['FCH'], d['VNC']
    NPAIR = H // 2               # head pairs (even head rows 0-63, odd 64-127)
    FH = FCH // 2                # fc split point (two weight tiles)

    nc = bacc.Bacc("TRN2", target_bir_lowering=False, debug=False, num_devices=8)

    # ---- dram parameters ----
    h0_p = nc.declare_dram_parameter("h0", [128, ECH, T], F32, isOutput=False)
    wqkv_p = nc.declare_dram_parameter("wqkv", [L, 128, ECH, 3 * E], BF16, isOutput=False)
    wproj_p = nc.declare_dram_parameter("wproj", [L, 128, ECH, E], BF16, isOutput=False)
    wfc_p = nc.declare_dram_parameter("wfc", [L, 128, ECH, F], BF16, isOutput=False)
    wfc2_p = nc.declare_dram_parameter("wfc2", [L, 128, FCH, E], BF16, isOutput=False)
    wlm_p = nc.declare_dram_parameter("wlm", [128, ECH, VNC * 512], BF16, isOutput=False)
    masks_p = nc.declare_dram_parameter("masks", [2, 128, 2, 128], BF16, isOutput=False)
    if has_bias['qkv']:
        bqk_p = nc.declare_dram_parameter("bqk", [L, 2 * ECH, 128, 1], F32, isOutput=False)
    if has_bias['v']:
        bv_p = nc.declare_dram_parameter("bv", [L, 1, E], BF16, isOutput=False)
    if has_bias['proj']:
        bproj_p = nc.declare_dram_parameter("bproj", [L, ECH, 128, 1], F32, isOutput=False)
    if has_bias['fc']:
        bfc_p = nc.declare_dram_parameter("bfc", [L, FCH, 128, 1], F32, isOutput=False)
    if has_bias['fc2']:
        bfc2_p = nc.declare_dram_parameter("bfc2", [L, ECH, 128, 1], F32, isOutput=False)
    if has_bias['lm']:
        blm_p = nc.declare_dram_parameter("blm", [1, VNC * 512], BF16, isOutput=False)
    out_p = nc.declare_dram_parameter("logits", [T, V], F32, isOutput=True)

    with TileContext(nc) as tc:
        with (
            tc.tile_pool(name="persist", bufs=1) as persist,
            tc.tile_pool(name="acts", bufs=1) as acts,
            tc.tile_pool(name="wpool", bufs=2) as wpool,
            tc.tile_pool(name="stage", bufs=3) as stage,
            tc.tile_pool(name="stage2", bufs=2) as stage2,
            tc.tile_pool(name="small", bufs=2) as small,
            tc.tile_pool(name="psMM", bufs=2, space="PSUM") as psMM,
            tc.tile_pool(name="psY", bufs=2, space="PSUM") as psY,
            tc.tile_pool(name="dramcc", bufs=2, space="DRAM") as dcc,
        ):
            # ---- persistent tiles ----
            h_sb = persist.tile([128, ECH, T], F32, tag="h")
            nc.sync.dma_start(h_sb[:], h0_p.ap())
            masks_sb = persist.tile([128, 2, 2, 128], BF16, tag="masks")
            nc.sync.dma_start(masks_sb[:], masks_p.ap().rearrange("p s h u -> s p h u"))
            ones_sb = persist.tile([128, 128], BF16, tag="ones")
            nc.gpsimd.memset(ones_sb[:], 1.0)
            neginv_sb = persist.tile([1, 128], BF16, tag="neginv")
            nc.gpsimd.memset(neginv_sb[:], -1.0 / E)
            eps_sb = persist.tile([1, 1], F32, tag="eps")
            nc.gpsimd.memset(eps_sb[:], 1e-5)
            v_aug = persist.tile([128, 2 * QCH, H, 65], BF16, tag="vaug")
            nc.gpsimd.memset(v_aug[:, :, :, 64:65], 1.0)

            def layernorm(out_tile):
                """h_sb (feature-major f32) -> out_tile [128, ECH, T] bf16 normalized."""
                s12 = psY.tile([1, 2, T], F32, tag="yps")  # s1 bank0, s2 bank1
                for kc in range(ECH):
                    h16 = stage.tile([128, T], BF16, tag="ln_h16")
                    if kc == ECH - 1:  # keep the serial tail off the slow gpsimd cast
                        nc.vector.tensor_copy(out=h16[:], in_=h_sb[:, kc, :])
                    else:
                        nc.gpsimd.tensor_copy(out=h16[:], in_=h_sb[:, kc, :])
                    sq = stage.tile([128, T], BF16, tag="ln_sq")
                    nc.vector.tensor_tensor(sq[:], h16[:], h16[:], ALU.mult)
                    nc.tensor.matmul(s12[0:1, 0, :], ones_sb[:, 0:1], h16[:],
                                     start=(kc == 0), stop=(kc == ECH - 1))
                    nc.tensor.matmul(s12[0:1, 1, :], ones_sb[:, 0:1], sq[:],
                                     start=(kc == 0), stop=(kc == ECH - 1))
                # row chain: rstd = exp(-0.5*ln(var+eps)); mean bcast via -1/E column
                ssb = small.tile([1, 2, T], F32, tag="ln_ssb")
                nc.vector.tensor_copy(out=ssb[:], in_=s12[:])
                m16 = small.tile([1, T], BF16, tag="ln_m16")
                nc.vector.tensor_copy(out=m16[:], in_=ssb[0:1, 0, :])
                q = small.tile([1, T], F32, tag="ln_tmp")
                nc.vector.tensor_tensor(q[:], ssb[0:1, 0, :], ssb[0:1, 0, :], ALU.mult)
                v2 = small.tile([1, T], F32, tag="ln_tmp")
                nc.vector.scalar_tensor_tensor(v2[:], q[:], -1.0 / E, ssb[0:1, 1, :],
                                               ALU.mult, ALU.add)
                lnv = small.tile([1, T], F32, tag="ln_tmp")
                nc.scalar.activation(lnv[:], v2[:], AF.Ln, bias=eps_sb[:], scale=1.0 / E)
                r16 = small.tile([1, T], BF16, tag="ln_r16")
                nc.scalar.activation(r16[:], lnv[:], AF.Exp, scale=-0.5)
                # broadcast rows to [128, T]: -mean via -1/E column x s1, rstd via ones
                mbp = psY.tile([128, T], F32, tag="yps")
                nc.tensor.matmul(mbp[:], neginv_sb[:], m16[:], start=True, stop=True)
                rbp = psY.tile([128, T], F32, tag="yps")
                nc.tensor.matmul(rbp[:], ones_sb[0:1, :], r16[:], start=True, stop=True)
                for kc in range(ECH):
                    t1 = stage.tile([128, T], F32, tag="ln_t1")
                    nc.vector.tensor_tensor(t1[:], h_sb[:, kc, :], mbp[:], ALU.add)
                    nc.vector.tensor_tensor(out_tile[:, kc, :], t1[:], rbp[:], ALU.mult)

            for l in range(L):
                # ---------------- ln1 -> aT (bf16 feature-major) ----------------
                aT = acts.tile([128, ECH, T], BF16, tag="lnout")
                layernorm(aT)

                # ship aT to the pair; compute q locally meanwhile
                cc_in = dcc.tile([E * T], BF16, tag="cc_in")
                cc_out = dcc.tile([2, E * T], BF16, tag="cc_out")
                cc_in_v = cc_in[:].rearrange("(p q t) -> p q t", p=128, q=ECH)
                for kc in range(ECH):
                    nc.sync.dma_start(cc_in_v[:, kc, :], aT[:, kc, :])
                nc.gpsimd.collective_compute(
                    "AllGather", ALU.bypass,
                    replica_groups=[[0, 1], [2, 3], [4, 5], [6, 7]],
                    ins=[cc_in[:]], outs=[cc_out[:]])

                wqk = wpool.tile([128, ECH, 2 * E], BF16, tag="W")
                nc.sync.dma_start(wqk[:], wqkv_p[l, :, :, 0:2 * E])
                wv = wpool.tile([128, ECH, E], BF16, tag="W")
                nc.sync.dma_start(wv[:], wqkv_p[l, :, :, 2 * E:3 * E])
                if has_bias['qkv']:
                    bqk_sb = small.tile([128, 2 * ECH], F32, tag="bqk")
                    nc.sync.dma_start(bqk_sb[:], bqk_p[l].rearrange("c p one -> p (c one)"))
                if has_bias['v']:
                    bv_sb = small.tile([1, E], BF16, tag="bv")
                    nc.sync.dma_start(bv_sb[:], bv_p[l])

                # q chunks (local aT only) — fills the collective gap
                qT = acts.tile([128, ECH, T], BF16, tag="qT")
                for mc in range(ECH):
                    ps = psMM.tile([128, T], F32, tag="mm")
                    for kc in range(ECH):
                        nc.tensor.matmul(ps[:], wqk[:, kc, 128 * mc:128 * (mc + 1)],
                                         aT[:, kc, :], start=(kc == 0), stop=(kc == ECH - 1))
                    if has_bias['qkv']:
                        nc.vector.tensor_scalar_add(qT[:, mc, :], ps[:], bqk_sb[:, mc:mc + 1])
                    else:
                        nc.vector.tensor_copy(out=qT[:, mc, :], in_=ps[:])

                # gathered aT (both pars; par index == core order in pair)
                aT_all = acts.tile([128, ECH, 2, T], BF16, tag="aT_all")
                for par in range(2):
                    nc.sync.dma_start(
                        aT_all[:, :, par, :],
                        cc_out[par].rearrange("(p q t) -> p q t", p=128, q=ECH))

                # k (feature-major) and v (token-major) for both pars
                kT_all = acts.tile([128, ECH, 2, T], BF16, tag="kTall")
                for par in range(2):
                    for mc in range(ECH):
                        ps = psMM.tile([128, T], F32, tag="mm")
                        for kc in range(ECH):
                            nc.tensor.matmul(ps[:], wqk[:, kc, E + 128 * mc:E + 128 * (mc + 1)],
                                             aT_all[:, kc, par, :],
                                             start=(kc == 0), stop=(kc == ECH - 1))
                        if has_bias['qkv']:
                            nc.vector.tensor_scalar_add(kT_all[:, mc, par, :], ps[:],
                                                        bqk_sb[:, ECH + mc:ECH + mc + 1])
                        else:
                            nc.vector.tensor_copy(out=kT_all[:, mc, par, :], in_=ps[:])
                NW = E // 2
                HH = NW // 64  # heads per half
                for par in range(2):
                    for t in range(QCH):
                        for nn in range(2):
                            ps = psMM.tile([128, NW], F32, tag="mm")
                            for kc in range(ECH):
                                nc.tensor.matmul(ps[:], aT_all[:, kc, par, 128 * t:128 * (t + 1)],
                                                 wv[:, kc, nn * NW:(nn + 1) * NW],
                                                 start=(kc == 0), stop=(kc == ECH - 1 and not has_bias['v']))
                            if has_bias['v']:
                                nc.tensor.matmul(ps[:], ones_sb[0:1, 0:128],
                                                 bv_sb[0:1, nn * NW:(nn + 1) * NW],
                                                 start=False, stop=True)
                            blk = par * QCH + t
                            nc.vector.tensor_copy(
                                out=v_aug[:, blk, nn * HH:(nn + 1) * HH, 0:64],
                                in_=ps[:].rearrange("p (h dd) -> p h dd", h=HH))

                # prefetch proj weights during attention (DMA only)
                wp = wpool.tile([128, ECH, E], BF16, tag="W")
                nc.sync.dma_start(wp[:], wproj_p[l])

                # ---------------- attention ----------------
                yT_c = acts.tile([128, ECH, T], BF16, tag="yTc")
                for qp in range(NPAIR):
                    yps = psY.tile([128, 2, T], F32, tag="yps")
                    first = True
                    for ccn in range(QCH):
                        qlo = 128 * ccn
                        for par in range(2):
                            blk = par * QCH + ccn
                            aps = psMM.tile([128, 2, T], F32, tag="mm")
                            for h01 in range(2):
                                plo = 64 * h01
                                nc.tensor.matmul(
                                    aps[:, h01, qlo:T],
                                    kT_all[plo:plo + 64, qp, par, 128 * ccn:128 * (ccn + 1)],
                                    qT[plo:plo + 64, qp, qlo:T],
                                    start=True, stop=True)
                            att = stage.tile([128, 2, T], BF16, tag="attsb")
                            nc.scalar.activation(att[:, :, qlo:T], aps[:, :, qlo:T], AF.Exp)
                            nc.gpsimd.tensor_tensor(
                                att[:, :, qlo:qlo + 128], att[:, :, qlo:qlo + 128],
                                masks_sb[:, par, :, :], ALU.mult)
                            for h01 in range(2):
                                nc.tensor.matmul(yps[0:65, h01, qlo:T],
                                                 v_aug[:, blk, 2 * qp + h01, :],
                                                 att[:, h01, qlo:T],
                                                 start=first, stop=(ccn == QCH - 1 and par == 1))
                            first = False
                    # normalize: y *= 1/denom (row 64) via exp(-ln(d)) on the scalar
                    # engine (same table set as the attention exp; DVE reciprocal
                    # is ~3.4us/row), odd head shifted via DMA
                    ln_d = small.tile([65, 2, T], F32, tag="recl")
                    nc.scalar.activation(ln_d[64:65, :, :], yps[64:65, :, :], AF.Ln)
                    rec = small.tile([65, 2, T], BF16, tag="rec")
                    nc.scalar.activation(rec[64:65, :, :], ln_d[64:65, :, :], AF.Exp,
                                         scale=-1.0)
                    for h01 in range(2):
                        bps = psMM.tile([64, T], F32, tag="mm")
                        nc.tensor.matmul(bps[:], ones_sb[64:65, 0:64], rec[64:65, h01, :],
                                         start=True, stop=True)
                        bcast = stage.tile([64, T], BF16, tag="bcast")
                        nc.vector.tensor_copy(out=bcast[:], in_=bps[:])
                        if h01 == 0:
                            nc.vector.tensor_tensor(yT_c[0:64, qp, :], yps[0:64, 0, :],
                                                    bcast[:], ALU.mult)
                        else:
                            ystg = stage.tile([64, T], BF16, tag="ystg")
                            nc.vector.tensor_tensor(ystg[:], yps[0:64, 1, :], bcast[:], ALU.mult)
                            nc.sync.dma_start(yT_c[64:128, qp, :], ystg[:])

                # ---------------- proj + residual ----------------
                if has_bias['proj']:
                    bproj_sb = small.tile([128, ECH], F32, tag="bproj")
                    nc.sync.dma_start(bproj_sb[:], bproj_p[l].rearrange("c p one -> p (c one)"))
                for oc in range(ECH):
                    ps = psMM.tile([128, T], F32, tag="mm")
                    for kc in range(ECH):
                        nc.tensor.matmul(ps[:], wp[:, kc, 128 * oc:128 * (oc + 1)],
                                         yT_c[:, kc, :], start=(kc == 0), stop=(kc == ECH - 1))
                    if has_bias['proj']:
                        nc.vector.tensor_scalar_add(ps[:], ps[:], bproj_sb[:, oc:oc + 1])
                    hs = h_sb[:, oc, :]
                    nc.vector.tensor_tensor(hs, hs, ps[:], ALU.add)

                # ---------------- ln2 -> mT ----------------
                mT = acts.tile([128, ECH, T], BF16, tag="lnout")
                layernorm(mT)

                # ---------------- fc1 + gelu (two weight halves) ----------------
                if has_bias['fc']:
                    bfc_sb = small.tile([128, FCH], F32, tag="bfc")
                    nc.sync.dma_start(bfc_sb[:], bfc_p[l].rearrange("c p one -> p (c one)"))
                gT = acts.tile([128, FCH, T], BF16, tag="gT")
                for half in range(2):
                    wf = wpool.tile([128, ECH, FH * 128], BF16, tag="W")
                    nc.sync.dma_start(wf[:], wfc_p[l, :, :, half * FH * 128:(half + 1) * FH * 128])
                    for fi in range(FH):
                        fm = half * FH + fi
                        ps = psMM.tile([128, T], F32, tag="mm")
                        for kc in range(ECH):
                            nc.tensor.matmul(ps[:], wf[:, kc, 128 * fi:128 * (fi + 1)],
                                             mT[:, kc, :], start=(kc == 0), stop=(kc == ECH - 1))
                        bias_arg = bfc_sb[:, fm:fm + 1] if has_bias['fc'] else 0.0
                        nc.scalar.activation(gT[:, fm, :], ps[:], AF.Gelu_apprx_tanh, bias=bias_arg)

                # ---------------- fc2 + residual (two passes over kc halves) ----------------
                if has_bias['fc2']:
                    bfc2_sb = small.tile([128, ECH], F32, tag="bfc2")
                    nc.sync.dma_start(bfc2_sb[:], bfc2_p[l].rearrange("c p one -> p (c one)"))
                for half in range(2):
                    wf2 = wpool.tile([128, FH, E], BF16, tag="W")
                    nc.sync.dma_start(wf2[:], wfc2_p[l, :, half * FH:(half + 1) * FH, :])
                    for oc in range(ECH):
                        ps = psMM.tile([128, T], F32, tag="mm")
                        for ki in range(FH):
                            kc = half * FH + ki
                            nc.tensor.matmul(ps[:], wf2[:, ki, 128 * oc:128 * (oc + 1)],
                                             gT[:, kc, :], start=(ki == 0), stop=(ki == FH - 1))
                        if has_bias['fc2'] and half == 0:
                            nc.vector.tensor_scalar_add(ps[:], ps[:], bfc2_sb[:, oc:oc + 1])
                        hs = h_sb[:, oc, :]
                        nc.vector.tensor_tensor(hs, hs, ps[:], ALU.add)

            # ---------------- final ln + lm head ----------------
            hfT = acts.tile([128, ECH, T], BF16, tag="lnout")
            layernorm(hfT)
            if has_bias['lm']:
                blm_sb = small.tile([1, VNC * 512], BF16, tag="blm")
                nc.sync.dma_start(blm_sb[:], blm_p[:])
            for n in range(VNC):
                wl = wpool.tile([128, ECH, 512], BF16, tag="Wlm")
                nc.sync.dma_start(wl[:], wlm_p[:, :, 512 * n:512 * (n + 1)])
                NWv = min(512, V - 512 * n)
                TG = 2 if QCH % 2 == 0 else 1  # t-chunks staged per output DMA
                for th in range(QCH // TG):
                    lstg = stage2.tile([128, TG, 512], F32, tag="lmstg")
                    for ti in range(TG):
                        t = TG * th + ti
                        ps = psMM.tile([128, 512], F32, tag="mm")
                        for kc in range(ECH):
                            nc.tensor.matmul(ps[:], hfT[:, kc, 128 * t:128 * (t + 1)],
                                             wl[:, kc, :],
                                             start=(kc == 0), stop=(kc == ECH - 1 and not has_bias['lm']))
                        if has_bias['lm']:
                            nc.tensor.matmul(ps[:], ones_sb[0:1, 0:128],
                                             blm_sb[0:1, 512 * n:512 * (n + 1)],
                                             start=False, stop=True)
                        nc.vector.tensor_copy(out=lstg[:, ti, :], in_=ps[:])
                    nc.sync.dma_start(
                        out_p.ap()[128 * TG * th:128 * TG * (th + 1), 512 * n:512 * n + NWv]
                        .rearrange("(t p) n -> p t n", p=128),
                        lstg[:, :, 0:NWv])
    return nc


# ---------------------------------------------------------------------------
# host prep
# ---------------------------------------------------------------------------

def host_prep(inputs, c):
    d = derived(c)
    B, S, L, H, D, F, V, E, T = c['B'], c['S'], c['L'], c['H'], c['D'], c['F'], c['V'], d['E'], d['T']
    ECH, FCH, QCH, KCH, VNC = d['ECH'], d['FCH'], d['QCH'], d['KCH'], d['VNC']

    f32 = lambda a: np.asarray(a, np.float32)
    x = np.asarray(inputs['x']).astype(np.int64)
    wte, wpe = f32(inputs['wte']), f32(inputs['wpe'])
    g1, b1 = f32(inputs['ln1_g']), f32(inputs['ln1_b'])
    aw, ab = f32(inputs['attn_w']), f32(inputs['attn_b'])
    pw, pb = f32(inputs['attn_proj_w']), f32(inputs['attn_proj_b'])
    g2, b2 = f32(inputs['ln2_g']), f32(inputs['ln2_b'])
    fw, fb = f32(inputs['fc_w']), f32(inputs['fc_b'])
    p2w, p2b = f32(inputs['fc_proj_w']), f32(inputs['fc_proj_b'])
    gf, bf_ = f32(inputs['lnf_g']), f32(inputs['lnf_b'])
    lm = f32(inputs['lm_head_w'])

    scale = 1.0 / np.sqrt(D)
    # fold ln1 gamma/beta into attn_w/attn_b ; scale q by 1/sqrt(D)
    aw_f = aw * g1[:, :, None]              # [L, E, 3E]
    ab_f = ab + np.einsum('le,lef->lf', b1, aw)
    aw_f[:, :, :E] *= scale
    ab_f[:, :E] *= scale
    fw_f = fw * g2[:, :, None]
    fb_f = fb + np.einsum('le,lef->lf', b2, fw)
    lm_f = lm * gf[:, None]
    blm_f = bf_ @ lm                         # [V]

    def bfc16(a):
        return np.ascontiguousarray(a).astype(BF)

    wqkv = bfc16(aw_f.reshape(L, ECH, 128, 3 * E).transpose(0, 2, 1, 3))
    wproj = bfc16(pw.reshape(L, ECH, 128, E).transpose(0, 2, 1, 3))
    wfc = bfc16(fw_f.reshape(L, ECH, 128, F).transpose(0, 2, 1, 3))
    wfc2 = bfc16(p2w.reshape(L, FCH, 128, E).transpose(0, 2, 1, 3))
    wlm_pad = np.zeros((E, VNC * 512), np.float32)
    wlm_pad[:, :V] = lm_f
    wlm = bfc16(wlm_pad.reshape(ECH, 128, VNC * 512).transpose(1, 0, 2))

    has_bias = dict(
        qkv=bool(np.any(ab_f[:, :2 * E])), v=bool(np.any(ab_f[:, 2 * E:])),
        proj=bool(np.any(pb)), fc=bool(np.any(fb_f)), fc2=bool(np.any(p2b)),
        lm=bool(np.any(blm_f)))

    # masks [2(par p), 128(key slot s), 2(head dup), 128(query u')]:
    # key block (p,c) slot s = global 256c + 2s + p; query u' -> global 2(128c+u')+P
    # valid iff 2s + p <= 2u' + P
    def diag_masks(P):
        s = np.arange(128)
        u = np.arange(128)
        ms = []
        for p in range(2):
            m = (2 * s[:, None] + p) <= (2 * u[None, :] + P)
            ms.append(np.stack([m, m], axis=1))  # duplicate for the head dim
        return np.stack(ms).astype(BF)           # [2, 128, 2, 128]

    # embeddings, strided, feature-major [128, ECH, T]
    emb = wte[x] + wpe[:S][None, :, :]       # [B, S, E] f32
    in_maps = []
    metas = []
    for core in range(8):
        b, p = core // 2, core % 2
        hT = np.ascontiguousarray(emb[b, p::2, :].T)          # [E, T]
        h0 = np.ascontiguousarray(
            hT.reshape(ECH, 128, T).transpose(1, 0, 2)).astype(np.float32)
        m = dict(h0=h0, wqkv=wqkv, wproj=wproj, wfc=wfc, wfc2=wfc2, wlm=wlm,
                 masks=diag_masks(p))
        if has_bias['qkv']:
            m['bqk'] = np.ascontiguousarray(
                ab_f[:, :2 * E].reshape(L, 2 * ECH, 128, 1)).astype(np.float32)
        if has_bias['v']:
            m['bv'] = ab_f[:, 2 * E:].reshape(L, 1, E).astype(BF)
        if has_bias['proj']:
            m['bproj'] = pb.reshape(L, ECH, 128, 1).astype(np.float32)
        if has_bias['fc']:
            m['bfc'] = fb_f.reshape(L, FCH, 128, 1).astype(np.float32)
        if has_bias['fc2']:
            m['bfc2'] = p2b.reshape(L, ECH, 128, 1).astype(np.float32)
        if has_bias['lm']:
            blm_pad = np.zeros((1, VNC * 512), np.float32)
            blm_pad[0, :V] = blm_f
            m['blm'] = blm_pad.astype(BF)
        in_maps.append(m)
        metas.append((b, p))
    return in_maps, metas, has_bias


def run(inputs, c, nc=None, has_bias=None, in_maps=None, metas=None, want_raw=False, trace=False):
    if in_maps is None:
        in_maps, metas, has_bias = host_prep(inputs, c)
    if nc is None:
        nc = build(c, has_bias)
        nc.compile()
    res = run_bass_kernel_spmd(nc, in_maps, core_ids=list(range(8)), trace=trace)
    B, S, V = c['B'], c['S'], c['V']
    out = np.empty((B, S, V), np.float32)
    for core in range(8):
        b, p = metas[core]
        out[b, p::2, :] = res.results[core]["logits"]
    if want_raw:
        return out, nc, res
    return out, nc


# ---------------------------------------------------------------------------
# harness entry point: kernel(**inputs) -> full logits [B, S, V] float32
# ---------------------------------------------------------------------------
_NC_CACHE = {}


def kernel(**inputs):
    c = cfg_full()
    in_maps, metas, has_bias = host_prep(inputs, c)
    key = tuple(sorted(has_bias.items()))
    if key not in _NC_CACHE:
        nc = build(c, has_bias)
        nc.compile()
        _NC_CACHE[key] = nc
    nc = _NC_CACHE[key]
    res = run_bass_kernel_spmd(nc, in_maps, core_ids=list(range(8)))
    B, S, V = c['B'], c['S'], c['V']
    out = np.empty((B, S, V), np.float32)
    for core in range(8):
        b, p = metas[core]
        out[b, p::2, :] = res.results[core]["logits"]
    return out


# revision 13
# speedup vs baseline: 1.2010x; 1.2010x over previous
"""GPT-2 forward on 8 TRN2 NeuronCores — feature-major context-parallel Bass/Tile kernel.

Sharding: 4 sequences x 2 cores each (strided interleave: core 2b+p owns tokens
of sequence b at global positions {2u+p}). Per layer each pair AllGathers its
LN1 output aT (bf16, feature-major, E*T = 768KB) and every core computes k/v
for the FULL sequence locally — cheaper than exchanging k+v and it removes the
small-descriptor re-interleave unpack entirely. Key/value blocks stay PAR-major
(block (p,c) = par p's local tokens [128c,128c+128)); causality is encoded in
two host-built diagonal masks.

Residual h is FEATURE-major f32 in SBUF for the whole kernel: no DMA-transposes.
LayerNorm stats are computed with PE ones-matmuls over bf16 copies of h (sum)
and h^2 (sumsq); rstd = exp(-0.5*ln(var+eps)) so the scalar engine stays in the
natural_log_exp table set shared with the attention exp (2 table switches per
layer, for gelu, same as the token-major version). Mean/rstd rows are broadcast
to [128,T] with rank-1 PE matmuls (ones / -1/E columns) and the normalize TTs
read them straight from PSUM.

Attention: scores for head pairs are packed into disjoint PE row groups
(K=64 at base partitions 0/64) writing one 2-bank PSUM tile, exp'd in a single
ACT op, masked in a single DVE op; av is causally trimmed; softmax denominators
come from a ones-column appended to v. Odd-head outputs are normalized at
partitions 0-63 and shifted to 64-127 with one small SBUF-SBUF DMA per pair.

LN gamma/beta and the 1/sqrt(D) scale are folded into weights on the host;
biases that are identically zero are skipped at build time.
"""
import sys, os
sys.path.insert(0, '/opt/trn_rl_repo')
import numpy as np
import ml_dtypes
import concourse.bass as bass
import concourse.mybir as mybir
from concourse import bacc
from concourse.bass_utils import run_bass_kernel_spmd
from concourse.tile import TileContext

F32 = mybir.dt.float32
BF16 = mybir.dt.bfloat16
AF = mybir.ActivationFunctionType
ALU = mybir.AluOpType
BF = ml_dtypes.bfloat16


def cfg_full():
    return dict(B=4, S=1024, L=12, H=12, D=64, F=3072, V=50257)


def cfg_mini():
    return dict(B=4, S=256, L=2, H=2, D=64, F=256, V=640)


def derived(c):
    d = dict(c)
    d['E'] = c['H'] * c['D']
    d['T'] = c['S'] // 2          # local tokens per core
    d['QCH'] = d['T'] // 128      # query/token chunks (T/128)
    d['KCH'] = c['S'] // 128      # global key chunks
    d['ECH'] = d['E'] // 128      # embed chunks
    d['FCH'] = c['F'] // 128      # mlp hidden chunks
    d['VNC'] = (c['V'] + 511) // 512  # lm-head n-chunks
    assert d['T'] % 128 == 0 and d['E'] % 128 == 0 and c['F'] % 128 == 0
    return d


def build(c, has_bias):
    """has_bias: dict of bools (qkv, v, proj, fc, fc2, lm) — ops skipped when zero."""
    d = derived(c)
    T, E, H, D, F, V, L = d['T'], d['E'], c['H'], c['D'], c['F'], c['V'], c['L']
    QCH, KCH, ECH, FCH, VNC = d['QCH'], d['KCH'], d['ECH'], d['FCH'], d['VNC']
    NPAIR = H // 2               # head pairs (even head rows 0-63, odd 64-127)
    FH = FCH // 2                # fc split point (two weight tiles)

    nc = bacc.Bacc("TRN2", target_bir_lowering=False, debug=False, num_devices=8)

    # ---- dram parameters ----
    h0_p = nc.declare_dram_parameter("h0", [128, ECH, T], F32, isOutput=False)
    wqkv_p = nc.declare_dram_parameter("wqkv", [L, 128, ECH, 3 * E], BF16, isOutput=False)
    wproj_p = nc.declare_dram_parameter("wproj", [L, 128, ECH, E], BF16, isOutput=False)
    wfc_p = nc.declare_dram_parameter("wfc", [L, 128, ECH, F], BF16, isOutput=False)
    wfc2_p = nc.declare_dram_parameter("wfc2", [L, 128, FCH, E], BF16, isOutput=False)
    wlm_p = nc.declare_dram_parameter("wlm", [128, ECH, VNC * 512], BF16, isOutput=False)
    masks_p = nc.declare_dram_parameter("masks", [2, 128, 2, 128], BF16, isOutput=False)
    if has_bias['qkv']:
        bqk_p = nc.declare_dram_parameter("bqk", [L, 2 * ECH, 128, 1], F32, isOutput=False)
    if has_bias['v']:
        bv_p = nc.declare_dram_parameter("bv", [L, 1, E], BF16, isOutput=False)
    if has_bias['proj']:
        bproj_p = nc.declare_dram_parameter("bproj", [L, ECH, 128, 1], F32, isOutput=False)
    if has_bias['fc']:
        bfc_p = nc.declare_dram_parameter("bfc", [L, FCH, 128, 1], F32, isOutput=False)
    if has_bias['fc2']:
        bfc2_p = nc.declare_dram_parameter("bfc2", [L, ECH, 128, 1], F32, isOutput=False)
    if has_bias['lm']:
        blm_p = nc.declare_dram_parameter("blm", [1, VNC * 512], BF16, isOutput=False)
    out_p = nc.declare_dram_parameter("logits", [T, V], F32, isOutput=True)

    with TileContext(nc) as tc:
        with (
            tc.tile_pool(name="persist", bufs=1) as persist,
            tc.tile_pool(name="acts", bufs=1) as acts,
            tc.tile_pool(name="wpool", bufs=2) as wpool,
            tc.tile_pool(name="stage", bufs=3) as stage,
            tc.tile_pool(name="stage2", bufs=2) as stage2,
            tc.tile_pool(name="small", bufs=2) as small,
            tc.tile_pool(name="psMM", bufs=2, space="PSUM") as psMM,
            tc.tile_pool(name="psY", bufs=2, space="PSUM") as psY,
            tc.tile_pool(name="dramcc", bufs=2, space="DRAM") as dcc,
        ):
            # ---- persistent tiles ----
            h_sb = persist.tile([128, ECH, T], F32, tag="h")
            nc.sync.dma_start(h_sb[:], h0_p.ap())
            masks_sb = persist.tile([128, 2, 2, 128], BF16, tag="masks")
            nc.sync.dma_start(masks_sb[:], masks_p.ap().rearrange("p s h u -> s p h u"))
            ones_sb = persist.tile([128, 128], BF16, tag="ones")
            nc.gpsimd.memset(ones_sb[:], 1.0)
            neginv_sb = persist.tile([1, 128], BF16, tag="neginv")
            nc.gpsimd.memset(neginv_sb[:], -1.0 / E)
            eps_sb = persist.tile([1, 1], F32, tag="eps")
            nc.gpsimd.memset(eps_sb[:], 1e-5)
            v_aug = persist.tile([128, 2 * QCH, H, 65], BF16, tag="vaug")
            nc.gpsimd.memset(v_aug[:, :, :, 64:65], 1.0)

            def layernorm(out_tile):
                """h_sb (feature-major f32) -> out_tile [128, ECH, T] bf16 normalized."""
                s12 = psY.tile([1, 2, T], F32, tag="yps")  # s1 bank0, s2 bank1
                for kc in range(ECH):
                    h16 = stage.tile([128, T], BF16, tag="ln_h16")
                    if kc == ECH - 1:  # keep the serial tail off the slow gpsimd cast
                        nc.vector.tensor_copy(out=h16[:], in_=h_sb[:, kc, :])
                    else:
                        nc.gpsimd.tensor_copy(out=h16[:], in_=h_sb[:, kc, :])
                    sq = stage.tile([128, T], BF16, tag="ln_sq")
                    nc.vector.tensor_tensor(sq[:], h16[:], h16[:], ALU.mult)
                    nc.tensor.matmul(s12[0:1, 0, :], ones_sb[:, 0:1], h16[:],
                                     start=(kc == 0), stop=(kc == ECH - 1))
                    nc.tensor.matmul(s12[0:1, 1, :], ones_sb[:, 0:1], sq[:],
                                     start=(kc == 0), stop=(kc == ECH - 1))
                # row chain: rstd = exp(-0.5*ln(var+eps)); mean bcast via -1/E column
                ssb = small.tile([1, 2, T], F32, tag="ln_ssb")
                nc.vector.tensor_copy(out=ssb[:], in_=s12[:])
                m16 = small.tile([1, T], BF16, tag="ln_m16")
                nc.vector.tensor_copy(out=m16[:], in_=ssb[0:1, 0, :])
                q = small.tile([1, T], F32, tag="ln_tmp")
                nc.vector.tensor_tensor(q[:], ssb[0:1, 0, :], ssb[0:1, 0, :], ALU.mult)
                v2 = small.tile([1, T], F32, tag="ln_tmp")
                nc.vector.scalar_tensor_tensor(v2[:], q[:], -1.0 / E, ssb[0:1, 1, :],
                                               ALU.mult, ALU.add)
                lnv = small.tile([1, T], F32, tag="ln_tmp")
                nc.scalar.activation(lnv[:], v2[:], AF.Ln, bias=eps_sb[:], scale=1.0 / E)
                r16 = small.tile([1, T], BF16, tag="ln_r16")
                nc.scalar.activation(r16[:], lnv[:], AF.Exp, scale=-0.5)
                # broadcast rows to [128, T]: -mean via -1/E column x s1, rstd via ones
                mbp = psY.tile([128, T], F32, tag="yps")
                nc.tensor.matmul(mbp[:], neginv_sb[:], m16[:], start=True, stop=True)
                rbp = psY.tile([128, T], F32, tag="yps")
                nc.tensor.matmul(rbp[:], ones_sb[0:1, :], r16[:], start=True, stop=True)
                for kc in range(ECH):
                    t1 = stage.tile([128, T], F32, tag="ln_t1")
                    nc.vector.tensor_tensor(t1[:], h_sb[:, kc, :], mbp[:], ALU.add)
                    nc.vector.tensor_tensor(out_tile[:, kc, :], t1[:], rbp[:], ALU.mult)

            for l in range(L):
                # ---------------- ln1 -> aT (bf16 feature-major) ----------------
                aT = acts.tile([128, ECH, T], BF16, tag="lnout")
                layernorm(aT)

                # ship aT to the pair; compute q locally meanwhile
                cc_in = dcc.tile([E * T], BF16, tag="cc_in")
                cc_out = dcc.tile([2, E * T], BF16, tag="cc_out")
                cc_in_v = cc_in[:].rearrange("(p q t) -> p q t", p=128, q=ECH)
                for kc in range(ECH):
                    nc.sync.dma_start(cc_in_v[:, kc, :], aT[:, kc, :])
                nc.gpsimd.collective_compute(
                    "AllGather", ALU.bypass,
                    replica_groups=[[0, 1], [2, 3], [4, 5], [6, 7]],
                    ins=[cc_in[:]], outs=[cc_out[:]])

                wqk = wpool.tile([128, ECH, 2 * E], BF16, tag="W")
                nc.sync.dma_start(wqk[:], wqkv_p[l, :, :, 0:2 * E])
                wv = wpool.tile([128, ECH, E], BF16, tag="W")
                nc.sync.dma_start(wv[:], wqkv_p[l, :, :, 2 * E:3 * E])
                if has_bias['qkv']:
                    bqk_sb = small.tile([128, 2 * ECH], F32, tag="bqk")
                    nc.sync.dma_start(bqk_sb[:], bqk_p[l].rearrange("c p one -> p (c one)"))
                if has_bias['v']:
                    bv_sb = small.tile([1, E], BF16, tag="bv")
                    nc.sync.dma_start(bv_sb[:], bv_p[l])

                # q chunks (local aT only) — fills the collective gap
                qT = acts.tile([128, ECH, T], BF16, tag="qT")
                for mc in range(ECH):
                    ps = psMM.tile([128, T], F32, tag="mm")
                    for kc in range(ECH):
                        nc.tensor.matmul(ps[:], wqk[:, kc, 128 * mc:128 * (mc + 1)],
                                         aT[:, kc, :], start=(kc == 0), stop=(kc == ECH - 1))
                    if has_bias['qkv']:
                        nc.vector.tensor_scalar_add(qT[:, mc, :], ps[:], bqk_sb[:, mc:mc + 1])
                    else:
                        nc.vector.tensor_copy(out=qT[:, mc, :], in_=ps[:])

                # gathered aT (both pars; par index == core order in pair)
                aT_all = acts.tile([128, ECH, 2, T], BF16, tag="aT_all")
                for par in range(2):
                    nc.sync.dma_start(
                        aT_all[:, :, par, :],
                        cc_out[par].rearrange("(p q t) -> p q t", p=128, q=ECH))

                # k (feature-major) and v (token-major) for both pars
                kT_all = acts.tile([128, ECH, 2, T], BF16, tag="kTall")
                for par in range(2):
                    for mc in range(ECH):
                        ps = psMM.tile([128, T], F32, tag="mm")
                        for kc in range(ECH):
                            nc.tensor.matmul(ps[:], wqk[:, kc, E + 128 * mc:E + 128 * (mc + 1)],
                                             aT_all[:, kc, par, :],
                                             start=(kc == 0), stop=(kc == ECH - 1))
                        if has_bias['qkv']:
                            nc.vector.tensor_scalar_add(kT_all[:, mc, par, :], ps[:],
                                                        bqk_sb[:, ECH + mc:ECH + mc + 1])
                        else:
                            nc.vector.tensor_copy(out=kT_all[:, mc, par, :], in_=ps[:])
                NW = E // 2
                HH = NW // 64  # heads per half
                for par in range(2):
                    for t in range(QCH):
                        for nn in range(2):
                            ps = psMM.tile([128, NW], F32, tag="mm")
                            for kc in range(ECH):
                                nc.tensor.matmul(ps[:], aT_all[:, kc, par, 128 * t:128 * (t + 1)],
                                                 wv[:, kc, nn * NW:(nn + 1) * NW],
                                                 start=(kc == 0), stop=(kc == ECH - 1 and not has_bias['v']))
                            if has_bias['v']:
                                nc.tensor.matmul(ps[:], ones_sb[0:1, 0:128],
                                                 bv_sb[0:1, nn * NW:(nn + 1) * NW],
                                                 start=False, stop=True)
                            blk = par * QCH + t
                            nc.vector.tensor_copy(
                                out=v_aug[:, blk, nn * HH:(nn + 1) * HH, 0:64],
                                in_=ps[:].rearrange("p (h dd) -> p h dd", h=HH))

                # prefetch proj weights during attention (DMA only)
                wp = wpool.tile([128, ECH, E], BF16, tag="W")
                nc.sync.dma_start(wp[:], wproj_p[l])

                # ---------------- attention ----------------
                # Pair qp's softmax-normalization is emitted AFTER pair qp+1's
                # block matmuls so the PE's in-order stream never stalls on the
                # ACT denominator chain (head-of-line blocking).
                yT_c = acts.tile([128, ECH, T], BF16, tag="yTc")

                def attn_blocks(qp):
                    yps = psY.tile([128, 2, T], F32, tag="yps")
                    first = True
                    for ccn in range(QCH):
                        qlo = 128 * ccn
                        for par in range(2):
                            blk = par * QCH + ccn
                            aps = psMM.tile([128, 2, T], F32, tag="mm")
                            for h01 in range(2):
                                plo = 64 * h01
                                nc.tensor.matmul(
                                    aps[:, h01, qlo:T],
                                    kT_all[plo:plo + 64, qp, par, 128 * ccn:128 * (ccn + 1)],
                                    qT[plo:plo + 64, qp, qlo:T],
                                    start=True, stop=True)
                            att = stage.tile([128, 2, T], BF16, tag="attsb")
                            nc.scalar.activation(att[:, :, qlo:T], aps[:, :, qlo:T], AF.Exp)
                            nc.vector.tensor_tensor(
                                att[:, :, qlo:qlo + 128], att[:, :, qlo:qlo + 128],
                                masks_sb[:, par, :, :], ALU.mult)
                            for h01 in range(2):
                                nc.tensor.matmul(yps[0:65, h01, qlo:T],
                                                 v_aug[:, blk, 2 * qp + h01, :],
                                                 att[:, h01, qlo:T],
                                                 start=first, stop=(ccn == QCH - 1 and par == 1))
                            first = False
                    return yps

                def attn_norm(qp, yps):
                    # y *= 1/denom (row 64) via exp(-ln(d)) on the scalar engine
                    # (same table set as the attention exp; DVE reciprocal is
                    # ~3.4us/row); odd head shifted to partitions 64-127 via DMA
                    ln_d = small.tile([65, 2, T], F32, tag="recl")
                    nc.scalar.activation(ln_d[64:65, :, :], yps[64:65, :, :], AF.Ln)
                    rec = small.tile([65, 2, T], BF16, tag="rec")
                    nc.scalar.activation(rec[64:65, :, :], ln_d[64:65, :, :], AF.Exp,
                                         scale=-1.0)
                    for h01 in range(2):
                        bps = psMM.tile([64, T], F32, tag="mm")
                        nc.tensor.matmul(bps[:], ones_sb[64:65, 0:64], rec[64:65, h01, :],
                                         start=True, stop=True)
                        bcast = stage.tile([64, T], BF16, tag="bcast")
                        nc.vector.tensor_copy(out=bcast[:], in_=bps[:])
                        if h01 == 0:
                            nc.vector.tensor_tensor(yT_c[0:64, qp, :], yps[0:64, 0, :],
                                                    bcast[:], ALU.mult)
                        else:
                            ystg = stage.tile([64, T], BF16, tag="ystg")
                            nc.vector.tensor_tensor(ystg[:], yps[0:64, 1, :], bcast[:], ALU.mult)
                            nc.sync.dma_start(yT_c[64:128, qp, :], ystg[:])

                prev = None
                for qp in range(NPAIR):
                    yps = attn_blocks(qp)
                    if prev is not None:
                        attn_norm(prev[0], prev[1])
                    prev = (qp, yps)
                attn_norm(prev[0], prev[1])

                # ---------------- proj + residual ----------------
                if has_bias['proj']:
                    bproj_sb = small.tile([128, ECH], F32, tag="bproj")
                    nc.sync.dma_start(bproj_sb[:], bproj_p[l].rearrange("c p one -> p (c one)"))
                for oc in range(ECH):
                    ps = psMM.tile([128, T], F32, tag="mm")
                    for kc in range(ECH):
                        nc.tensor.matmul(ps[:], wp[:, kc, 128 * oc:128 * (oc + 1)],
                                         yT_c[:, kc, :], start=(kc == 0), stop=(kc == ECH - 1))
                    if has_bias['proj']:
                        nc.vector.tensor_scalar_add(ps[:], ps[:], bproj_sb[:, oc:oc + 1])
                    hs = h_sb[:, oc, :]
                    nc.vector.tensor_tensor(hs, hs, ps[:], ALU.add)

                # ---------------- ln2 -> mT ----------------
                mT = acts.tile([128, ECH, T], BF16, tag="lnout")
                layernorm(mT)

                # ---------------- fc1 + gelu (two weight halves) ----------------
                if has_bias['fc']:
                    bfc_sb = small.tile([128, FCH], F32, tag="bfc")
                    nc.sync.dma_start(bfc_sb[:], bfc_p[l].rearrange("c p one -> p (c one)"))
                gT = acts.tile([128, FCH, T], BF16, tag="gT")
                for half in range(2):
                    wf = wpool.tile([128, ECH, FH * 128], BF16, tag="W")
                    nc.sync.dma_start(wf[:], wfc_p[l, :, :, half * FH * 128:(half + 1) * FH * 128])
                    for fi in range(FH):
                        fm = half * FH + fi
                        ps = psMM.tile([128, T], F32, tag="mm")
                        for kc in range(ECH):
                            nc.tensor.matmul(ps[:], wf[:, kc, 128 * fi:128 * (fi + 1)],
                                             mT[:, kc, :], start=(kc == 0), stop=(kc == ECH - 1))
                        bias_arg = bfc_sb[:, fm:fm + 1] if has_bias['fc'] else 0.0
                        nc.scalar.activation(gT[:, fm, :], ps[:], AF.Gelu_apprx_tanh, bias=bias_arg)

                # ---------------- fc2 + residual (two passes over kc halves) ----------------
                if has_bias['fc2']:
                    bfc2_sb = small.tile([128, ECH], F32, tag="bfc2")
                    nc.sync.dma_start(bfc2_sb[:], bfc2_p[l].rearrange("c p one -> p (c one)"))
                for half in range(2):
                    wf2 = wpool.tile([128, FH, E], BF16, tag="W")
                    nc.sync.dma_start(wf2[:], wfc2_p[l, :, half * FH:(half + 1) * FH, :])
                    for oc in range(ECH):
                        ps = psMM.tile([128, T], F32, tag="mm")
                        for ki in range(FH):
                            kc = half * FH + ki
                            nc.tensor.matmul(ps[:], wf2[:, ki, 128 * oc:128 * (oc + 1)],
                                             gT[:, kc, :], start=(ki == 0), stop=(ki == FH - 1))
                        if has_bias['fc2'] and half == 0:
                            nc.vector.tensor_scalar_add(ps[:], ps[:], bfc2_sb[:, oc:oc + 1])
                        hs = h_sb[:, oc, :]
                        nc.vector.tensor_tensor(hs, hs, ps[:], ALU.add)

            # ---------------- final ln + lm head ----------------
            hfT = acts.tile([128, ECH, T], BF16, tag="lnout")
            layernorm(hfT)
            if has_bias['lm']:
                blm_sb = small.tile([1, VNC * 512], BF16, tag="blm")
                nc.sync.dma_start(blm_sb[:], blm_p[:])
            for n in range(VNC):
                wl = wpool.tile([128, ECH, 512], BF16, tag="Wlm")
                nc.sync.dma_start(wl[:], wlm_p[:, :, 512 * n:512 * (n + 1)])
                NWv = min(512, V - 512 * n)
                TG = 2 if QCH % 2 == 0 else 1  # t-chunks staged per output DMA
                for th in range(QCH // TG):
                    lstg = stage2.tile([128, TG, 512], F32, tag="lmstg")
                    for ti in range(TG):
                        t = TG * th + ti
                        ps = psMM.tile([128, 512], F32, tag="mm")
                        for kc in range(ECH):
                            nc.tensor.matmul(ps[:], hfT[:, kc, 128 * t:128 * (t + 1)],
                                             wl[:, kc, :],
                                             start=(kc == 0), stop=(kc == ECH - 1 and not has_bias['lm']))
                        if has_bias['lm']:
                            nc.tensor.matmul(ps[:], ones_sb[0:1, 0:128],
                                             blm_sb[0:1, 512 * n:512 * (n + 1)],
                                             start=False, stop=True)
                        nc.vector.tensor_copy(out=lstg[:, ti, :], in_=ps[:])
                    nc.sync.dma_start(
                        out_p.ap()[128 * TG * th:128 * TG * (th + 1), 512 * n:512 * n + NWv]
                        .rearrange("(t p) n -> p t n", p=128),
                        lstg[:, :, 0:NWv])
    return nc


# ---------------------------------------------------------------------------
# host prep
# ---------------------------------------------------------------------------

def host_prep(inputs, c):
    d = derived(c)
    B, S, L, H, D, F, V, E, T = c['B'], c['S'], c['L'], c['H'], c['D'], c['F'], c['V'], d['E'], d['T']
    ECH, FCH, QCH, KCH, VNC = d['ECH'], d['FCH'], d['QCH'], d['KCH'], d['VNC']

    f32 = lambda a: np.asarray(a, np.float32)
    x = np.asarray(inputs['x']).astype(np.int64)
    wte, wpe = f32(inputs['wte']), f32(inputs['wpe'])
    g1, b1 = f32(inputs['ln1_g']), f32(inputs['ln1_b'])
    aw, ab = f32(inputs['attn_w']), f32(inputs['attn_b'])
    pw, pb = f32(inputs['attn_proj_w']), f32(inputs['attn_proj_b'])
    g2, b2 = f32(inputs['ln2_g']), f32(inputs['ln2_b'])
    fw, fb = f32(inputs['fc_w']), f32(inputs['fc_b'])
    p2w, p2b = f32(inputs['fc_proj_w']), f32(inputs['fc_proj_b'])
    gf, bf_ = f32(inputs['lnf_g']), f32(inputs['lnf_b'])
    lm = f32(inputs['lm_head_w'])

    scale = 1.0 / np.sqrt(D)
    # fold ln1 gamma/beta into attn_w/attn_b ; scale q by 1/sqrt(D)
    aw_f = aw * g1[:, :, None]              # [L, E, 3E]
    ab_f = ab + np.einsum('le,lef->lf', b1, aw)
    aw_f[:, :, :E] *= scale
    ab_f[:, :E] *= scale
    fw_f = fw * g2[:, :, None]
    fb_f = fb + np.einsum('le,lef->lf', b2, fw)
    lm_f = lm * gf[:, None]
    blm_f = bf_ @ lm                         # [V]

    def bfc16(a):
        return np.ascontiguousarray(a).astype(BF)

    wqkv = bfc16(aw_f.reshape(L, ECH, 128, 3 * E).transpose(0, 2, 1, 3))
    wproj = bfc16(pw.reshape(L, ECH, 128, E).transpose(0, 2, 1, 3))
    wfc = bfc16(fw_f.reshape(L, ECH, 128, F).transpose(0, 2, 1, 3))
    wfc2 = bfc16(p2w.reshape(L, FCH, 128, E).transpose(0, 2, 1, 3))
    wlm_pad = np.zeros((E, VNC * 512), np.float32)
    wlm_pad[:, :V] = lm_f
    wlm = bfc16(wlm_pad.reshape(ECH, 128, VNC * 512).transpose(1, 0, 2))

    has_bias = dict(
        qkv=bool(np.any(ab_f[:, :2 * E])), v=bool(np.any(ab_f[:, 2 * E:])),
        proj=bool(np.any(pb)), fc=bool(np.any(fb_f)), fc2=bool(np.any(p2b)),
        lm=bool(np.any(blm_f)))

    # masks [2(par p), 128(key slot s), 2(head dup), 128(query u')]:
    # key block (p,c) slot s = global 256c + 2s + p; query u' -> global 2(128c+u')+P
    # valid iff 2s + p <= 2u' + P
    def diag_masks(P):
        s = np.arange(128)
        u = np.arange(128)
        ms = []
        for p in range(2):
            m = (2 * s[:, None] + p) <= (2 * u[None, :] + P)
            ms.append(np.stack([m, m], axis=1))  # duplicate for the head dim
        return np.stack(ms).astype(BF)           # [2, 128, 2, 128]

    # embeddings, strided, feature-major [128, ECH, T]
    emb = wte[x] + wpe[:S][None, :, :]       # [B, S, E] f32
    in_maps = []
    metas = []
    for core in range(8):
        b, p = core // 2, core % 2
        hT = np.ascontiguousarray(emb[b, p::2, :].T)          # [E, T]
        h0 = np.ascontiguousarray(
            hT.reshape(ECH, 128, T).transpose(1, 0, 2)).astype(np.float32)
        m = dict(h0=h0, wqkv=wqkv, wproj=wproj, wfc=wfc, wfc2=wfc2, wlm=wlm,
                 masks=diag_masks(p))
        if has_bias['qkv']:
            m['bqk'] = np.ascontiguousarray(
                ab_f[:, :2 * E].reshape(L, 2 * ECH, 128, 1)).astype(np.float32)
        if has_bias['v']:
            m['bv'] = ab_f[:, 2 * E:].reshape(L, 1, E).astype(BF)
        if has_bias['proj']:
            m['bproj'] = pb.reshape(L, ECH, 128, 1).astype(np.float32)
        if has_bias['fc']:
            m['bfc'] = fb_f.reshape(L, FCH, 128, 1).astype(np.float32)
        if has_bias['fc2']:
            m['bfc2'] = p2b.reshape(L, ECH, 128, 1).astype(np.float32)
        if has_bias['lm']:
            blm_pad = np.zeros((1, VNC * 512), np.float32)
            blm_pad[0, :V] = blm_f
            m['blm'] = blm_pad.astype(BF)
        in_maps.append(m)
        metas.append((b, p))
    return in_maps, metas, has_bias


def run(inputs, c, nc=None, has_bias=None, in_maps=None, metas=None, want_raw=False, trace=False):
    if in_maps is None:
        in_maps, metas, has_bias = host_prep(inputs, c)
    if nc is None:
        nc = build(c, has_bias)
        nc.compile()
    res = run_bass_kernel_spmd(nc, in_maps, core_ids=list(range(8)), trace=trace)
    B, S, V = c['B'], c['S'], c['V']
    out = np.empty((B, S, V), np.float32)
    for core in range(8):
        b, p = metas[core]
        out[b, p::2, :] = res.results[core]["logits"]
    if want_raw:
        return out, nc, res
    return out, nc


# ---------------------------------------------------------------------------
# harness entry point: kernel(**inputs) -> full logits [B, S, V] float32
# ---------------------------------------------------------------------------
_NC_CACHE = {}


def kernel(**inputs):
    c = cfg_full()
    in_maps, metas, has_bias = host_prep(inputs, c)
    key = tuple(sorted(has_bias.items()))
    if key not in _NC_CACHE:
        nc = build(c, has_bias)
        nc.compile()
        _NC_CACHE[key] = nc
    nc = _NC_CACHE[key]
    res = run_bass_kernel_spmd(nc, in_maps, core_ids=list(range(8)))
    B, S, V = c['B'], c['S'], c['V']
    out = np.empty((B, S, V), np.float32)
    for core in range(8):
        b, p = metas[core]
        out[b, p::2, :] = res.results[core]["logits"]
    return out


# revision 16
# speedup vs baseline: 1.2115x; 1.0087x over previous
"""GPT-2 forward on 8 TRN2 NeuronCores — feature-major context-parallel Bass/Tile kernel.

Sharding: 4 sequences x 2 cores each (strided interleave: core 2b+p owns tokens
of sequence b at global positions {2u+p}). Per layer each pair AllGathers its
LN1 output aT (bf16, feature-major, E*T = 768KB) and every core computes k/v
for the FULL sequence locally — cheaper than exchanging k+v and it removes the
small-descriptor re-interleave unpack entirely. Key/value blocks stay PAR-major
(block (p,c) = par p's local tokens [128c,128c+128)); causality is encoded in
two host-built diagonal masks.

Residual h is FEATURE-major f32 in SBUF for the whole kernel: no DMA-transposes.
LayerNorm stats are computed with PE ones-matmuls over bf16 copies of h (sum)
and h^2 (sumsq); rstd = exp(-0.5*ln(var+eps)) so the scalar engine stays in the
natural_log_exp table set shared with the attention exp (2 table switches per
layer, for gelu, same as the token-major version). Mean/rstd rows are broadcast
to [128,T] with rank-1 PE matmuls (ones / -1/E columns) and the normalize TTs
read them straight from PSUM.

Attention: scores for head pairs are packed into disjoint PE row groups
(K=64 at base partitions 0/64) writing one 2-bank PSUM tile, exp'd in a single
ACT op, masked in a single DVE op; av is causally trimmed; softmax denominators
come from a ones-column appended to v. Odd-head outputs are normalized at
partitions 0-63 and shifted to 64-127 with one small SBUF-SBUF DMA per pair.

LN gamma/beta and the 1/sqrt(D) scale are folded into weights on the host;
biases that are identically zero are skipped at build time.
"""
import sys, os
sys.path.insert(0, '/opt/trn_rl_repo')
import numpy as np
import ml_dtypes
import concourse.bass as bass
import concourse.mybir as mybir
from concourse import bacc
from concourse.bass_utils import run_bass_kernel_spmd
from concourse.tile import TileContext

F32 = mybir.dt.float32
BF16 = mybir.dt.bfloat16
AF = mybir.ActivationFunctionType
ALU = mybir.AluOpType
BF = ml_dtypes.bfloat16


def cfg_full():
    return dict(B=4, S=1024, L=12, H=12, D=64, F=3072, V=50257)


def cfg_mini():
    return dict(B=4, S=256, L=2, H=2, D=64, F=256, V=640)


def derived(c):
    d = dict(c)
    d['E'] = c['H'] * c['D']
    d['T'] = c['S'] // 2          # local tokens per core
    d['QCH'] = d['T'] // 128      # query/token chunks (T/128)
    d['KCH'] = c['S'] // 128      # global key chunks
    d['ECH'] = d['E'] // 128      # embed chunks
    d['FCH'] = c['F'] // 128      # mlp hidden chunks
    d['VNC'] = (c['V'] + 511) // 512  # lm-head n-chunks
    assert d['T'] % 128 == 0 and d['E'] % 128 == 0 and c['F'] % 128 == 0
    return d


def build(c, has_bias):
    """has_bias: dict of bools (qkv, v, proj, fc, fc2, lm) — ops skipped when zero."""
    d = derived(c)
    T, E, H, D, F, V, L = d['T'], d['E'], c['H'], c['D'], c['F'], c['V'], c['L']
    QCH, KCH, ECH, FCH, VNC = d['QCH'], d['KCH'], d['ECH'], d['FCH'], d['VNC']
    NPAIR = H // 2               # head pairs (even head rows 0-63, odd 64-127)
    FH = FCH // 2                # fc split point (two weight tiles)

    nc = bacc.Bacc("TRN2", target_bir_lowering=False, debug=False, num_devices=8)

    # ---- dram parameters ----
    h0_p = nc.declare_dram_parameter("h0", [128, ECH, T], F32, isOutput=False)
    wqkv_p = nc.declare_dram_parameter("wqkv", [L, 128, ECH, 3 * E], BF16, isOutput=False)
    wproj_p = nc.declare_dram_parameter("wproj", [L, 128, ECH, E], BF16, isOutput=False)
    wfc_p = nc.declare_dram_parameter("wfc", [L, 128, ECH, F], BF16, isOutput=False)
    wfc2_p = nc.declare_dram_parameter("wfc2", [L, 128, FCH, E], BF16, isOutput=False)
    wlm_p = nc.declare_dram_parameter("wlm", [128, ECH, VNC * 512], BF16, isOutput=False)
    masks_p = nc.declare_dram_parameter("masks", [2, 128, 2, 128], BF16, isOutput=False)
    if has_bias['qkv']:
        bqk_p = nc.declare_dram_parameter("bqk", [L, 2 * ECH, 128, 1], F32, isOutput=False)
    if has_bias['v']:
        bv_p = nc.declare_dram_parameter("bv", [L, 1, E], BF16, isOutput=False)
    if has_bias['proj']:
        bproj_p = nc.declare_dram_parameter("bproj", [L, ECH, 128, 1], F32, isOutput=False)
    if has_bias['fc']:
        bfc_p = nc.declare_dram_parameter("bfc", [L, FCH, 128, 1], F32, isOutput=False)
    if has_bias['fc2']:
        bfc2_p = nc.declare_dram_parameter("bfc2", [L, ECH, 128, 1], F32, isOutput=False)
    if has_bias['lm']:
        blm_p = nc.declare_dram_parameter("blm", [1, VNC * 512], BF16, isOutput=False)
    out_p = nc.declare_dram_parameter("logits", [T, V], F32, isOutput=True)

    with TileContext(nc) as tc:
        with (
            tc.tile_pool(name="persist", bufs=1) as persist,
            tc.tile_pool(name="acts", bufs=1) as acts,
            tc.tile_pool(name="wpool", bufs=2) as wpool,
            tc.tile_pool(name="wlmpool", bufs=3) as wlmpool,
            tc.tile_pool(name="stage", bufs=3) as stage,
            tc.tile_pool(name="stage2", bufs=2) as stage2,
            tc.tile_pool(name="small", bufs=2) as small,
            tc.tile_pool(name="psMM", bufs=2, space="PSUM") as psMM,
            tc.tile_pool(name="psY", bufs=2, space="PSUM") as psY,
            tc.tile_pool(name="dramcc", bufs=2, space="DRAM") as dcc,
        ):
            # ---- persistent tiles ----
            h_sb = persist.tile([128, ECH, T], F32, tag="h")
            nc.sync.dma_start(h_sb[:], h0_p.ap())
            masks_sb = persist.tile([128, 2, 2, 128], BF16, tag="masks")
            nc.sync.dma_start(masks_sb[:], masks_p.ap().rearrange("p s h u -> s p h u"))
            ones_sb = persist.tile([128, 128], BF16, tag="ones")
            nc.gpsimd.memset(ones_sb[:], 1.0)
            neginv_sb = persist.tile([1, 128], BF16, tag="neginv")
            nc.gpsimd.memset(neginv_sb[:], -1.0 / E)
            eps_sb = persist.tile([1, 1], F32, tag="eps")
            nc.gpsimd.memset(eps_sb[:], 1e-5)
            v_aug = persist.tile([128, 2 * QCH, H, 65], BF16, tag="vaug")
            nc.gpsimd.memset(v_aug[:, :, :, 64:65], 1.0)

            def layernorm(out_tile):
                """h_sb (feature-major f32) -> out_tile [128, ECH, T] bf16 normalized."""
                s12 = psY.tile([1, 2, T], F32, tag="yps")  # s1 bank0, s2 bank1
                for kc in range(ECH):
                    h16 = stage.tile([128, T], BF16, tag="ln_h16")
                    if kc == ECH - 1:  # keep the serial tail off the slow gpsimd cast
                        nc.vector.tensor_copy(out=h16[:], in_=h_sb[:, kc, :])
                    else:
                        nc.gpsimd.tensor_copy(out=h16[:], in_=h_sb[:, kc, :])
                    sq = stage.tile([128, T], BF16, tag="ln_sq")
                    nc.vector.tensor_tensor(sq[:], h16[:], h16[:], ALU.mult)
                    nc.tensor.matmul(s12[0:1, 0, :], ones_sb[:, 0:1], h16[:],
                                     start=(kc == 0), stop=(kc == ECH - 1))
                    nc.tensor.matmul(s12[0:1, 1, :], ones_sb[:, 0:1], sq[:],
                                     start=(kc == 0), stop=(kc == ECH - 1))
                # row chain: rstd = exp(-0.5*ln(var+eps)); mean bcast via -1/E column
                ssb = small.tile([1, 2, T], F32, tag="ln_ssb")
                nc.vector.tensor_copy(out=ssb[:], in_=s12[:])
                m16 = small.tile([1, T], BF16, tag="ln_m16")
                nc.vector.tensor_copy(out=m16[:], in_=ssb[0:1, 0, :])
                q = small.tile([1, T], F32, tag="ln_tmp")
                nc.vector.tensor_tensor(q[:], ssb[0:1, 0, :], ssb[0:1, 0, :], ALU.mult)
                v2 = small.tile([1, T], F32, tag="ln_tmp")
                nc.vector.scalar_tensor_tensor(v2[:], q[:], -1.0 / E, ssb[0:1, 1, :],
                                               ALU.mult, ALU.add)
                lnv = small.tile([1, T], F32, tag="ln_tmp")
                nc.scalar.activation(lnv[:], v2[:], AF.Ln, bias=eps_sb[:], scale=1.0 / E)
                r16 = small.tile([1, T], BF16, tag="ln_r16")
                nc.scalar.activation(r16[:], lnv[:], AF.Exp, scale=-0.5)
                # broadcast rows to [128, T]: -mean via -1/E column x s1, rstd via ones
                mbp = psY.tile([128, T], F32, tag="yps")
                nc.tensor.matmul(mbp[:], neginv_sb[:], m16[:], start=True, stop=True)
                rbp = psY.tile([128, T], F32, tag="yps")
                nc.tensor.matmul(rbp[:], ones_sb[0:1, :], r16[:], start=True, stop=True)
                for kc in range(ECH):
                    t1 = stage.tile([128, T], F32, tag="ln_t1")
                    nc.vector.tensor_tensor(t1[:], h_sb[:, kc, :], mbp[:], ALU.add)
                    nc.vector.tensor_tensor(out_tile[:, kc, :], t1[:], rbp[:], ALU.mult)

            for l in range(L):
                # ---------------- ln1 -> aT (bf16 feature-major) ----------------
                aT = acts.tile([128, ECH, T], BF16, tag="lnout")
                layernorm(aT)

                # ship aT to the pair; compute q locally meanwhile
                cc_in = dcc.tile([E * T], BF16, tag="cc_in")
                cc_out = dcc.tile([2, E * T], BF16, tag="cc_out")
                cc_in_v = cc_in[:].rearrange("(p q t) -> p q t", p=128, q=ECH)
                for kc in range(ECH):
                    nc.sync.dma_start(cc_in_v[:, kc, :], aT[:, kc, :])
                nc.gpsimd.collective_compute(
                    "AllGather", ALU.bypass,
                    replica_groups=[[0, 1], [2, 3], [4, 5], [6, 7]],
                    ins=[cc_in[:]], outs=[cc_out[:]])

                wqk = wpool.tile([128, ECH, 2 * E], BF16, tag="W")
                nc.sync.dma_start(wqk[:], wqkv_p[l, :, :, 0:2 * E])
                wv = wpool.tile([128, ECH, E], BF16, tag="W")
                nc.sync.dma_start(wv[:], wqkv_p[l, :, :, 2 * E:3 * E])
                if has_bias['qkv']:
                    bqk_sb = small.tile([128, 2 * ECH], F32, tag="bqk")
                    nc.sync.dma_start(bqk_sb[:], bqk_p[l].rearrange("c p one -> p (c one)"))
                if has_bias['v']:
                    bv_sb = small.tile([1, E], BF16, tag="bv")
                    nc.sync.dma_start(bv_sb[:], bv_p[l])

                # q chunks (local aT only) — fills the collective gap
                qT = acts.tile([128, ECH, T], BF16, tag="qT")
                for mc in range(ECH):
                    ps = psMM.tile([128, T], F32, tag="mm")
                    for kc in range(ECH):
                        nc.tensor.matmul(ps[:], wqk[:, kc, 128 * mc:128 * (mc + 1)],
                                         aT[:, kc, :], start=(kc == 0), stop=(kc == ECH - 1))
                    if has_bias['qkv']:
                        nc.vector.tensor_scalar_add(qT[:, mc, :], ps[:], bqk_sb[:, mc:mc + 1])
                    else:
                        nc.vector.tensor_copy(out=qT[:, mc, :], in_=ps[:])

                # gathered aT (both pars; par index == core order in pair)
                aT_all = acts.tile([128, ECH, 2, T], BF16, tag="aT_all")
                for par in range(2):
                    nc.sync.dma_start(
                        aT_all[:, :, par, :],
                        cc_out[par].rearrange("(p q t) -> p q t", p=128, q=ECH))

                # k (feature-major) and v (token-major) for both pars
                kT_all = acts.tile([128, ECH, 2, T], BF16, tag="kTall")
                for par in range(2):
                    for mc in range(ECH):
                        ps = psMM.tile([128, T], F32, tag="mm")
                        for kc in range(ECH):
                            nc.tensor.matmul(ps[:], wqk[:, kc, E + 128 * mc:E + 128 * (mc + 1)],
                                             aT_all[:, kc, par, :],
                                             start=(kc == 0), stop=(kc == ECH - 1))
                        if has_bias['qkv']:
                            nc.vector.tensor_scalar_add(kT_all[:, mc, par, :], ps[:],
                                                        bqk_sb[:, ECH + mc:ECH + mc + 1])
                        else:
                            nc.vector.tensor_copy(out=kT_all[:, mc, par, :], in_=ps[:])
                NW = E // 2
                HH = NW // 64  # heads per half
                for par in range(2):
                    for t in range(QCH):
                        for nn in range(2):
                            ps = psMM.tile([128, NW], F32, tag="mm")
                            for kc in range(ECH):
                                nc.tensor.matmul(ps[:], aT_all[:, kc, par, 128 * t:128 * (t + 1)],
                                                 wv[:, kc, nn * NW:(nn + 1) * NW],
                                                 start=(kc == 0), stop=(kc == ECH - 1 and not has_bias['v']))
                            if has_bias['v']:
                                nc.tensor.matmul(ps[:], ones_sb[0:1, 0:128],
                                                 bv_sb[0:1, nn * NW:(nn + 1) * NW],
                                                 start=False, stop=True)
                            blk = par * QCH + t
                            nc.vector.tensor_copy(
                                out=v_aug[:, blk, nn * HH:(nn + 1) * HH, 0:64],
                                in_=ps[:].rearrange("p (h dd) -> p h dd", h=HH))

                # prefetch proj weights during attention (DMA only)
                wp = wpool.tile([128, ECH, E], BF16, tag="W")
                nc.sync.dma_start(wp[:], wproj_p[l])

                # ---------------- attention ----------------
                # Pair qp's softmax-normalization is emitted AFTER pair qp+1's
                # block matmuls so the PE's in-order stream never stalls on the
                # ACT denominator chain (head-of-line blocking).
                yT_c = acts.tile([128, ECH, T], BF16, tag="yTc")

                def attn_blocks(qp):
                    yps = psY.tile([128, 2, T], F32, tag="yps")
                    first = True
                    for ccn in range(QCH):
                        qlo = 128 * ccn
                        for par in range(2):
                            blk = par * QCH + ccn
                            aps = psMM.tile([128, 2, T], F32, tag="mm")
                            for h01 in range(2):
                                plo = 64 * h01
                                nc.tensor.matmul(
                                    aps[:, h01, qlo:T],
                                    kT_all[plo:plo + 64, qp, par, 128 * ccn:128 * (ccn + 1)],
                                    qT[plo:plo + 64, qp, qlo:T],
                                    start=True, stop=True)
                            att = stage.tile([128, 2, T], BF16, tag="attsb")
                            nc.scalar.activation(att[:, :, qlo:T], aps[:, :, qlo:T], AF.Exp)
                            nc.vector.tensor_tensor(
                                att[:, :, qlo:qlo + 128], att[:, :, qlo:qlo + 128],
                                masks_sb[:, par, :, :], ALU.mult)
                            for h01 in range(2):
                                nc.tensor.matmul(yps[0:65, h01, qlo:T],
                                                 v_aug[:, blk, 2 * qp + h01, :],
                                                 att[:, h01, qlo:T],
                                                 start=first, stop=(ccn == QCH - 1 and par == 1))
                            first = False
                    return yps

                def attn_norm(qp, yps):
                    # y *= 1/denom (row 64) via exp(-ln(d)) on the scalar engine
                    # (same table set as the attention exp; DVE reciprocal is
                    # ~3.4us/row); odd head shifted to partitions 64-127 via DMA
                    ln_d = small.tile([65, 2, T], F32, tag="recl")
                    nc.scalar.activation(ln_d[64:65, :, :], yps[64:65, :, :], AF.Ln)
                    rec = small.tile([65, 2, T], BF16, tag="rec")
                    nc.scalar.activation(rec[64:65, :, :], ln_d[64:65, :, :], AF.Exp,
                                         scale=-1.0)
                    for h01 in range(2):
                        bps = psMM.tile([64, T], F32, tag="mm")
                        nc.tensor.matmul(bps[:], ones_sb[64:65, 0:64], rec[64:65, h01, :],
                                         start=True, stop=True)
                        bcast = stage.tile([64, T], BF16, tag="bcast")
                        nc.vector.tensor_copy(out=bcast[:], in_=bps[:])
                        if h01 == 0:
                            nc.vector.tensor_tensor(yT_c[0:64, qp, :], yps[0:64, 0, :],
                                                    bcast[:], ALU.mult)
                        else:
                            ystg = stage.tile([64, T], BF16, tag="ystg")
                            nc.vector.tensor_tensor(ystg[:], yps[0:64, 1, :], bcast[:], ALU.mult)
                            nc.sync.dma_start(yT_c[64:128, qp, :], ystg[:])

                prev = None
                for qp in range(NPAIR):
                    yps = attn_blocks(qp)
                    if prev is not None:
                        attn_norm(prev[0], prev[1])
                    prev = (qp, yps)
                attn_norm(prev[0], prev[1])

                # ---------------- proj + residual ----------------
                if has_bias['proj']:
                    bproj_sb = small.tile([128, ECH], F32, tag="bproj")
                    nc.sync.dma_start(bproj_sb[:], bproj_p[l].rearrange("c p one -> p (c one)"))
                for oc in range(ECH):
                    ps = psMM.tile([128, T], F32, tag="mm")
                    for kc in range(ECH):
                        nc.tensor.matmul(ps[:], wp[:, kc, 128 * oc:128 * (oc + 1)],
                                         yT_c[:, kc, :], start=(kc == 0), stop=(kc == ECH - 1))
                    if has_bias['proj']:
                        nc.vector.tensor_scalar_add(ps[:], ps[:], bproj_sb[:, oc:oc + 1])
                    hs = h_sb[:, oc, :]
                    nc.vector.tensor_tensor(hs, hs, ps[:], ALU.add)

                # ---------------- ln2 -> mT ----------------
                mT = acts.tile([128, ECH, T], BF16, tag="lnout")
                layernorm(mT)

                # ---------------- fc1 + gelu (two weight halves) ----------------
                if has_bias['fc']:
                    bfc_sb = small.tile([128, FCH], F32, tag="bfc")
                    nc.sync.dma_start(bfc_sb[:], bfc_p[l].rearrange("c p one -> p (c one)"))
                gT = acts.tile([128, FCH, T], BF16, tag="gT")
                for half in range(2):
                    wf = wpool.tile([128, ECH, FH * 128], BF16, tag="W")
                    nc.sync.dma_start(wf[:], wfc_p[l, :, :, half * FH * 128:(half + 1) * FH * 128])
                    for fi in range(FH):
                        fm = half * FH + fi
                        ps = psMM.tile([128, T], F32, tag="mm")
                        for kc in range(ECH):
                            nc.tensor.matmul(ps[:], wf[:, kc, 128 * fi:128 * (fi + 1)],
                                             mT[:, kc, :], start=(kc == 0), stop=(kc == ECH - 1))
                        bias_arg = bfc_sb[:, fm:fm + 1] if has_bias['fc'] else 0.0
                        nc.scalar.activation(gT[:, fm, :], ps[:], AF.Gelu_apprx_tanh, bias=bias_arg)

                # ---------------- fc2 + residual (two passes over kc halves) ----------------
                if has_bias['fc2']:
                    bfc2_sb = small.tile([128, ECH], F32, tag="bfc2")
                    nc.sync.dma_start(bfc2_sb[:], bfc2_p[l].rearrange("c p one -> p (c one)"))
                for half in range(2):
                    wf2 = wpool.tile([128, FH, E], BF16, tag="W")
                    nc.sync.dma_start(wf2[:], wfc2_p[l, :, half * FH:(half + 1) * FH, :])
                    for oc in range(ECH):
                        ps = psMM.tile([128, T], F32, tag="mm")
                        for ki in range(FH):
                            kc = half * FH + ki
                            nc.tensor.matmul(ps[:], wf2[:, ki, 128 * oc:128 * (oc + 1)],
                                             gT[:, kc, :], start=(ki == 0), stop=(ki == FH - 1))
                        if has_bias['fc2'] and half == 0:
                            nc.vector.tensor_scalar_add(ps[:], ps[:], bfc2_sb[:, oc:oc + 1])
                        hs = h_sb[:, oc, :]
                        nc.vector.tensor_tensor(hs, hs, ps[:], ALU.add)

            # ---------------- final ln + lm head ----------------
            hfT = acts.tile([128, ECH, T], BF16, tag="lnout")
            layernorm(hfT)
            if has_bias['lm']:
                blm_sb = small.tile([1, VNC * 512], BF16, tag="blm")
                nc.sync.dma_start(blm_sb[:], blm_p[:])
            for n in range(VNC):
                wl = wlmpool.tile([128, ECH, 512], BF16, tag="Wlm")
                nc.sync.dma_start(wl[:], wlm_p[:, :, 512 * n:512 * (n + 1)])
                NWv = min(512, V - 512 * n)
                TG = 2 if QCH % 2 == 0 else 1  # t-chunks staged per output DMA
                for th in range(QCH // TG):
                    lstg = stage2.tile([128, TG, 512], F32, tag="lmstg")
                    for ti in range(TG):
                        t = TG * th + ti
                        ps = psMM.tile([128, 512], F32, tag="mm")
                        for kc in range(ECH):
                            nc.tensor.matmul(ps[:], hfT[:, kc, 128 * t:128 * (t + 1)],
                                             wl[:, kc, :],
                                             start=(kc == 0), stop=(kc == ECH - 1 and not has_bias['lm']))
                        if has_bias['lm']:
                            nc.tensor.matmul(ps[:], ones_sb[0:1, 0:128],
                                             blm_sb[0:1, 512 * n:512 * (n + 1)],
                                             start=False, stop=True)
                        # scalar engine is idle during the lm head; keep DVE free
                        # and release the PSUM slot quickly
                        nc.scalar.activation(lstg[:, ti, :], ps[:], AF.Copy)
                    nc.sync.dma_start(
                        out_p.ap()[128 * TG * th:128 * TG * (th + 1), 512 * n:512 * n + NWv]
                        .rearrange("(t p) n -> p t n", p=128),
                        lstg[:, :, 0:NWv])
    return nc


# ---------------------------------------------------------------------------
# host prep
# ---------------------------------------------------------------------------

def host_prep(inputs, c):
    d = derived(c)
    B, S, L, H, D, F, V, E, T = c['B'], c['S'], c['L'], c['H'], c['D'], c['F'], c['V'], d['E'], d['T']
    ECH, FCH, QCH, KCH, VNC = d['ECH'], d['FCH'], d['QCH'], d['KCH'], d['VNC']

    f32 = lambda a: np.asarray(a, np.float32)
    x = np.asarray(inputs['x']).astype(np.int64)
    wte, wpe = f32(inputs['wte']), f32(inputs['wpe'])
    g1, b1 = f32(inputs['ln1_g']), f32(inputs['ln1_b'])
    aw, ab = f32(inputs['attn_w']), f32(inputs['attn_b'])
    pw, pb = f32(inputs['attn_proj_w']), f32(inputs['attn_proj_b'])
    g2, b2 = f32(inputs['ln2_g']), f32(inputs['ln2_b'])
    fw, fb = f32(inputs['fc_w']), f32(inputs['fc_b'])
    p2w, p2b = f32(inputs['fc_proj_w']), f32(inputs['fc_proj_b'])
    gf, bf_ = f32(inputs['lnf_g']), f32(inputs['lnf_b'])
    lm = f32(inputs['lm_head_w'])

    scale = 1.0 / np.sqrt(D)
    # fold ln1 gamma/beta into attn_w/attn_b ; scale q by 1/sqrt(D)
    aw_f = aw * g1[:, :, None]              # [L, E, 3E]
    ab_f = ab + np.einsum('le,lef->lf', b1, aw)
    aw_f[:, :, :E] *= scale
    ab_f[:, :E] *= scale
    fw_f = fw * g2[:, :, None]
    fb_f = fb + np.einsum('le,lef->lf', b2, fw)
    lm_f = lm * gf[:, None]
    blm_f = bf_ @ lm                         # [V]

    def bfc16(a):
        return np.ascontiguousarray(a).astype(BF)

    wqkv = bfc16(aw_f.reshape(L, ECH, 128, 3 * E).transpose(0, 2, 1, 3))
    wproj = bfc16(pw.reshape(L, ECH, 128, E).transpose(0, 2, 1, 3))
    wfc = bfc16(fw_f.reshape(L, ECH, 128, F).transpose(0, 2, 1, 3))
    wfc2 = bfc16(p2w.reshape(L, FCH, 128, E).transpose(0, 2, 1, 3))
    wlm_pad = np.zeros((E, VNC * 512), np.float32)
    wlm_pad[:, :V] = lm_f
    wlm = bfc16(wlm_pad.reshape(ECH, 128, VNC * 512).transpose(1, 0, 2))

    has_bias = dict(
        qkv=bool(np.any(ab_f[:, :2 * E])), v=bool(np.any(ab_f[:, 2 * E:])),
        proj=bool(np.any(pb)), fc=bool(np.any(fb_f)), fc2=bool(np.any(p2b)),
        lm=bool(np.any(blm_f)))

    # masks [2(par p), 128(key slot s), 2(head dup), 128(query u')]:
    # key block (p,c) slot s = global 256c + 2s + p; query u' -> global 2(128c+u')+P
    # valid iff 2s + p <= 2u' + P
    def diag_masks(P):
        s = np.arange(128)
        u = np.arange(128)
        ms = []
        for p in range(2):
            m = (2 * s[:, None] + p) <= (2 * u[None, :] + P)
            ms.append(np.stack([m, m], axis=1))  # duplicate for the head dim
        return np.stack(ms).astype(BF)           # [2, 128, 2, 128]

    # embeddings, strided, feature-major [128, ECH, T]
    emb = wte[x] + wpe[:S][None, :, :]       # [B, S, E] f32
    in_maps = []
    metas = []
    for core in range(8):
        b, p = core // 2, core % 2
        hT = np.ascontiguousarray(emb[b, p::2, :].T)          # [E, T]
        h0 = np.ascontiguousarray(
            hT.reshape(ECH, 128, T).transpose(1, 0, 2)).astype(np.float32)
        m = dict(h0=h0, wqkv=wqkv, wproj=wproj, wfc=wfc, wfc2=wfc2, wlm=wlm,
                 masks=diag_masks(p))
        if has_bias['qkv']:
            m['bqk'] = np.ascontiguousarray(
                ab_f[:, :2 * E].reshape(L, 2 * ECH, 128, 1)).astype(np.float32)
        if has_bias['v']:
            m['bv'] = ab_f[:, 2 * E:].reshape(L, 1, E).astype(BF)
        if has_bias['proj']:
            m['bproj'] = pb.reshape(L, ECH, 128, 1).astype(np.float32)
        if has_bias['fc']:
            m['bfc'] = fb_f.reshape(L, FCH, 128, 1).astype(np.float32)
        if has_bias['fc2']:
            m['bfc2'] = p2b.reshape(L, ECH, 128, 1).astype(np.float32)
        if has_bias['lm']:
            blm_pad = np.zeros((1, VNC * 512), np.float32)
            blm_pad[0, :V] = blm_f
            m['blm'] = blm_pad.astype(BF)
        in_maps.append(m)
        metas.append((b, p))
    return in_maps, metas, has_bias


def run(inputs, c, nc=None, has_bias=None, in_maps=None, metas=None, want_raw=False, trace=False):
    if in_maps is None:
        in_maps, metas, has_bias = host_prep(inputs, c)
    if nc is None:
        nc = build(c, has_bias)
        nc.compile()
    res = run_bass_kernel_spmd(nc, in_maps, core_ids=list(range(8)), trace=trace)
    B, S, V = c['B'], c['S'], c['V']
    out = np.empty((B, S, V), np.float32)
    for core in range(8):
        b, p = metas[core]
        out[b, p::2, :] = res.results[core]["logits"]
    if want_raw:
        return out, nc, res
    return out, nc


# ---------------------------------------------------------------------------
# harness entry point: kernel(**inputs) -> full logits [B, S, V] float32
# ---------------------------------------------------------------------------
_NC_CACHE = {}


def kernel(**inputs):
    c = cfg_full()
    in_maps, metas, has_bias = host_prep(inputs, c)
    key = tuple(sorted(has_bias.items()))
    if key not in _NC_CACHE:
        nc = build(c, has_bias)
        nc.compile()
        _NC_CACHE[key] = nc
    nc = _NC_CACHE[key]
    res = run_bass_kernel_spmd(nc, in_maps, core_ids=list(range(8)))
    B, S, V = c['B'], c['S'], c['V']
    out = np.empty((B, S, V), np.float32)
    for core in range(8):
        b, p = metas[core]
        out[b, p::2, :] = res.results[core]["logits"]
    return out
